# revision 8
# baseline (speedup 1.0000x reference)
"""Trainium2 Bass kernel for a binarized (1w1a) BasicBlock:

    out1 = hardtanh(BN1(binconv(x, w1)))          # BN in training mode (batch stats)
    out  = hardtanh(BN2(binconv(out1, w2)) + x)   # identity shortcut

binconv(x, w) = conv3x3(sign(x), sign(w), pad=1) * (SCALE / K)

Sharding: data-parallel over batch (4 images per core on 8 cores), weights
replicated.  BN batch statistics (per-channel sum and sum-of-squares) are
combined with a tiny cross-core AllReduce.

Implementation notes:
  - sign() values (+-1, 0) are exact in bf16, and the 3x3x256 conv
    accumulates integers |S| <= 2304 in fp32 PSUM, so the convolutions are
    bit-exact at full bf16 TensorE rate.  The SCALE/K factor commutes
    through BatchNorm and is folded into eps:  eps_eff = eps / (SCALE/K)^2.
  - conv3x3 = 9 shifted matmuls per 128-channel block, accumulated in PSUM.
    Activations live in SBUF as [128 ch, n, 58, 58] zero-padded images so
    every shift is just an access-pattern offset.
  - Per-channel statistics ride on the ScalarE (activation accum_out) while
    TensorE streams matmuls; the second BN input (conv1 output) is stored as
    exact integers in fp16 and re-binarized with a single fused
    sign(A*y + B) activation per tile.
"""

import numpy as np
import ml_dtypes

import concourse.bass as bass
import concourse.tile as tile
from concourse import bacc, mybir
from concourse import bass_utils

N_CORES = 8
N, C, H, W = 32, 256, 56, 56
NL = N // N_CORES          # images per core
HP, WP = H + 2, W + 2      # zero-padded spatial
CB = C // 128              # channel blocks (2)
HT = 8                     # output rows per tile
N_HT = H // HT             # 7 tiles per image
FREE = HT * W              # 448 matmul free dim
SCALE, K = 1.0, 2
EPS = 1e-5
ALPHA = SCALE / K
EPS_EFF = EPS / (ALPHA * ALPHA)
M_TOT = float(N * H * W)   # BN reduction count (global batch)

F32 = mybir.dt.float32
F16 = mybir.dt.float16
BF16 = mybir.dt.bfloat16
AF = mybir.ActivationFunctionType
ALU = mybir.AluOpType

_CACHE = {}


def _conv_phase(nc, tc, pools, xb, wts, y16, sum_cols, sq_cols):
    """One binarized conv3x3 over all local images + stats accumulation.

    xb:   [cb] list of [128, NL, HP, WP] bf16 padded/binarized inputs
    wts:  [cb] list of [128, 3, 3, C] bf16 weights ([ci, dy, dx, co] layout)
    y16:  [cb] list of [128, NL, H, W] f16 outputs (exact integer sums)
    sum_cols/sq_cols: [cb] list of [128, NL * N_HT] f32 per-tile partials
    """
    psum = pools["psum"]
    for n in range(NL):
        for ht in range(N_HT):
            h0 = ht * HT
            idx = n * N_HT + ht
            for cob in range(CB):
                pt = psum.tile([128, FREE], F32, tag="pt", name="pt")
                k = 0
                for cib in range(CB):
                    for dy in range(3):
                        for dx in range(3):
                            nc.tensor.matmul(
                                pt[:],
                                wts[cib][:, dy, dx, cob * 128:(cob + 1) * 128],
                                xb[cib][:, n, h0 + dy:h0 + dy + HT, dx:dx + W],
                                start=(k == 0),
                                stop=(k == 17),
                            )
                            k += 1
                # fp32 PSUM -> exact integers in fp16, fused per-channel sum
                nc.scalar.activation(
                    out=y16[cob][:, n, h0:h0 + HT, :],
                    in_=pt[:],
                    func=AF.Copy,
                    accum_out=sum_cols[cob][:, idx:idx + 1],
                )
                # in-place square, fused per-channel sum of squares
                nc.scalar.activation(
                    out=pt[:],
                    in_=pt[:],
                    func=AF.Square,
                    accum_out=sq_cols[cob][:, idx:idx + 1],
                )


def _bn_affine(nc, pools, gstats, gb, g_col, b_col, a_out, b_out):
    """Per-channel-block A/B:  A = g * rsqrt(var + eps_eff),  B = b - mean * A.

    gstats: [128, 4] globally-reduced (sum, sumsq) per channel block
    """
    small = pools["small"]
    epst = pools["epst"]
    for cob in range(CB):
        mean = small.tile([128, 1], F32, tag=f"mean{cob}", name=f"mean{cob}")
        ex2 = small.tile([128, 1], F32, tag=f"ex2{cob}", name=f"ex2{cob}")
        msq = small.tile([128, 1], F32, tag=f"msq{cob}", name=f"msq{cob}")
        var = small.tile([128, 1], F32, tag=f"var{cob}", name=f"var{cob}")
        rstd = small.tile([128, 1], F32, tag=f"rstd{cob}", name=f"rstd{cob}")
        nc.vector.tensor_scalar_mul(mean[:], gstats[:, 2 * cob:2 * cob + 1], 1.0 / M_TOT)
        nc.vector.tensor_scalar_mul(ex2[:], gstats[:, 2 * cob + 1:2 * cob + 2], 1.0 / M_TOT)
        # var = ex2 - mean^2
        nc.vector.tensor_mul(msq[:], mean[:], mean[:])
        nc.vector.tensor_sub(var[:], ex2[:], msq[:])
        # rstd = 1 / sqrt(var + eps_eff)
        nc.scalar.activation(out=rstd[:], in_=var[:], func=AF.Sqrt, bias=epst[:])
        nc.vector.reciprocal(rstd[:], rstd[:])
        # A = g * rstd ; B = b - mean * A
        nc.vector.tensor_mul(a_out[cob][:], gb[:, g_col + cob:g_col + cob + 1], rstd[:])
        nc.vector.tensor_mul(mean[:], mean[:], a_out[cob][:])
        nc.vector.tensor_sub(b_out[cob][:], gb[:, b_col + cob:b_col + cob + 1], mean[:])


def build():
    """Build + compile the per-core Bass program (SPMD, 8 cores)."""
    nc = bacc.Bacc("TRN2", target_bir_lowering=False, debug=False,
                   num_devices=N_CORES)

    x_in = nc.dram_tensor("x", [NL, C, H, W], F32, kind="ExternalInput").ap()
    w1_in = nc.dram_tensor("w1t", [3, 3, C, C], BF16, kind="ExternalInput").ap()
    w2_in = nc.dram_tensor("w2t", [3, 3, C, C], BF16, kind="ExternalInput").ap()
    gb_in = nc.dram_tensor("gb", [128, 8], F32, kind="ExternalInput").ap()
    out_d = nc.dram_tensor("out", [NL, C, H, W], F32, kind="ExternalOutput").ap()

    rg = [list(range(N_CORES))]

    with tile.TileContext(nc) as tc:
        import contextlib
        with contextlib.ExitStack() as ctx:
            consts = ctx.enter_context(tc.tile_pool(name="consts", bufs=1))
            xbp = ctx.enter_context(tc.tile_pool(name="xbp", bufs=1))
            y16p = ctx.enter_context(tc.tile_pool(name="y16p", bufs=1))
            stage = ctx.enter_context(tc.tile_pool(name="stage", bufs=3))
            youtp = ctx.enter_context(tc.tile_pool(name="youtp", bufs=2))
            statp = ctx.enter_context(tc.tile_pool(name="statp", bufs=1))
            small = ctx.enter_context(tc.tile_pool(name="small", bufs=1))
            psum = ctx.enter_context(tc.tile_pool(name="psum", bufs=4, space="PSUM"))
            dram = ctx.enter_context(tc.tile_pool(name="dram", bufs=1, space="DRAM"))
            epst = small.tile([128, 1], F32, tag="epst", name="epst")
            nc.vector.memset(epst[:], EPS_EFF)
            pools = {"psum": psum, "small": small, "epst": epst}

            # ---- dummy AllReduce: absorb first-collective setup cost under conv1
            dzero = small.tile([128, 1], F32, tag="dzero", name="dzero")
            nc.vector.memset(dzero[:], 0.0)
            d_in0 = dram.tile([128, 1], F32, tag="d_in0", name="d_in0")
            d_out0 = dram.tile([128, 1], F32, tag="d_out0", name="d_out0")
            nc.sync.dma_start(out=d_in0[:], in_=dzero[:])
            nc.gpsimd.collective_compute(
                "AllReduce", ALU.add, replica_groups=rg,
                ins=[d_in0.opt()], outs=[d_out0.opt()],
            )

            # ---- constants
            gb = consts.tile([128, 8], F32, tag="gb", name="gb")
            nc.sync.dma_start(out=gb[:], in_=gb_in[:])
            w1s, w2s = [], []
            for cib in range(CB):
                w1t = consts.tile([128, 3, 3, C], BF16, tag=f"w1_{cib}", name=f"w1_{cib}")
                nc.sync.dma_start(
                    out=w1t[:],
                    in_=w1_in[:, :, cib * 128:(cib + 1) * 128, :].rearrange(
                        "dy dx ci co -> ci dy dx co"))
                w1s.append(w1t)
                w2t = consts.tile([128, 3, 3, C], BF16, tag=f"w2_{cib}", name=f"w2_{cib}")
                nc.sync.dma_start(
                    out=w2t[:],
                    in_=w2_in[:, :, cib * 128:(cib + 1) * 128, :].rearrange(
                        "dy dx ci co -> ci dy dx co"))
                w2s.append(w2t)

            # ---- padded binarized activations (reused: conv1 input, then conv2 input)
            xb = []
            for cib in range(CB):
                t = xbp.tile([128, NL, HP, WP], BF16, tag=f"xb{cib}", name=f"xb{cib}")
                nc.gpsimd.memset(t[:], 0.0)
                xb.append(t)

            # ---- conv outputs as exact integers (reused for conv1 then conv2)
            y16 = [y16p.tile([128, NL, H, W], F16, tag=f"y16_{cob}", name=f"y16_{cob}") for cob in range(CB)]

            # ---- stats partials
            s1c = [statp.tile([128, NL * N_HT], F32, tag=f"s1c{c}", name=f"s1c{c}") for c in range(CB)]
            q1c = [statp.tile([128, NL * N_HT], F32, tag=f"q1c{c}", name=f"q1c{c}") for c in range(CB)]
            s2c = [statp.tile([128, NL * N_HT], F32, tag=f"s2c{c}", name=f"s2c{c}") for c in range(CB)]
            q2c = [statp.tile([128, NL * N_HT], F32, tag=f"q2c{c}", name=f"q2c{c}") for c in range(CB)]

            # ---- phase 0: load x, binarize into padded buffers
            for n in range(NL):
                for cib in range(CB):
                    xs = stage.tile([128, H, W], F32, tag="xstage", name="xstage")
                    nc.sync.dma_start(
                        out=xs[:], in_=x_in[n, cib * 128:(cib + 1) * 128, :, :])
                    nc.scalar.activation(
                        out=xb[cib][:, n, 1:H + 1, 1:W + 1], in_=xs[:], func=AF.Sign)

            # ---- conv1 + stats
            _conv_phase(nc, tc, pools, xb, w1s, y16, s1c, q1c)

            # ---- reduce + AllReduce stats 1
            st1 = small.tile([128, 4], F32, tag="st1", name="st1")
            for cob in range(CB):
                nc.vector.reduce_sum(st1[:, 2 * cob:2 * cob + 1], s1c[cob][:],
                                     axis=mybir.AxisListType.X)
                nc.vector.reduce_sum(st1[:, 2 * cob + 1:2 * cob + 2], q1c[cob][:],
                                     axis=mybir.AxisListType.X)
            d_in1 = dram.tile([128, 4], F32, tag="d_in1", name="d_in1")
            d_out1 = dram.tile([128, 4], F32, tag="d_out1", name="d_out1")
            nc.sync.dma_start(out=d_in1[:], in_=st1[:])
            nc.gpsimd.collective_compute(
                "AllReduce", ALU.add, replica_groups=rg,
                ins=[d_in1.opt()], outs=[d_out1.opt()],
            )
            gstats1 = small.tile([128, 4], F32, tag="gstats1", name="gstats1")
            nc.sync.dma_start(out=gstats1[:], in_=d_out1[:])

            a1 = [small.tile([128, 1], F32, tag=f"a1_{c}", name=f"a1_{c}") for c in range(CB)]
            b1 = [small.tile([128, 1], F32, tag=f"b1_{c}", name=f"b1_{c}") for c in range(CB)]
            _bn_affine(nc, pools, gstats1, gb, g_col=0, b_col=2, a_out=a1, b_out=b1)

            # ---- phase 2: out1 = sign(A1 * y1 + B1) into the padded buffers
            for n in range(NL):
                for ht in range(N_HT):
                    h0 = ht * HT
                    for cob in range(CB):
                        nc.scalar.activation(
                            out=xb[cob][:, n, h0 + 1:h0 + HT + 1, 1:W + 1],
                            in_=y16[cob][:, n, h0:h0 + HT, :],
                            func=AF.Sign,
                            scale=a1[cob][:],
                            bias=b1[cob][:],
                        )

            # ---- conv2 + stats (y16 overwritten with conv2 integer sums)
            _conv_phase(nc, tc, pools, xb, w2s, y16, s2c, q2c)

            # ---- reduce + AllReduce stats 2
            st2 = small.tile([128, 4], F32, tag="st2", name="st2")
            for cob in range(CB):
                nc.vector.reduce_sum(st2[:, 2 * cob:2 * cob + 1], s2c[cob][:],
                                     axis=mybir.AxisListType.X)
                nc.vector.reduce_sum(st2[:, 2 * cob + 1:2 * cob + 2], q2c[cob][:],
                                     axis=mybir.AxisListType.X)
            d_in2 = dram.tile([128, 4], F32, tag="d_in2", name="d_in2")
            d_out2 = dram.tile([128, 4], F32, tag="d_out2", name="d_out2")
            nc.sync.dma_start(out=d_in2[:], in_=st2[:])
            nc.gpsimd.collective_compute(
                "AllReduce", ALU.add, replica_groups=rg,
                ins=[d_in2.opt()], outs=[d_out2.opt()],
            )
            gstats2 = small.tile([128, 4], F32, tag="gstats2", name="gstats2")
            nc.sync.dma_start(out=gstats2[:], in_=d_out2[:])

            a2 = [small.tile([128, 1], F32, tag=f"a2_{c}", name=f"a2_{c}") for c in range(CB)]
            b2 = [small.tile([128, 1], F32, tag=f"b2_{c}", name=f"b2_{c}") for c in range(CB)]
            _bn_affine(nc, pools, gstats2, gb, g_col=4, b_col=6, a_out=a2, b_out=b2)

            # ---- final: out = clip(A2 * y2 + B2 + x, -1, 1)
            for n in range(NL):
                for cib in range(CB):
                    xres = stage.tile([128, H, W], F32, tag="xstage", name="xstage")
                    nc.sync.dma_start(
                        out=xres[:], in_=x_in[n, cib * 128:(cib + 1) * 128, :, :])
                    yout = youtp.tile([128, H, W], F32, tag="yout", name="yout")
                    nc.scalar.activation(
                        out=yout[:], in_=y16[cib][:, n, :, :], func=AF.Identity,
                        scale=a2[cib][:], bias=b2[cib][:])
                    nc.vector.tensor_add(yout[:], yout[:], xres[:])
                    nc.vector.tensor_scalar(
                        out=yout[:], in0=yout[:], scalar1=1.0, scalar2=-1.0,
                        op0=ALU.min, op1=ALU.max)
                    nc.sync.dma_start(
                        out=out_d[n, cib * 128:(cib + 1) * 128, :, :], in_=yout[:])

    nc.compile()
    return nc


def _prep_inputs(x, w1, g1, b1, w2, g2, b2):
    """Host-side sharding + weight layout. Returns per-core input maps."""
    x = np.ascontiguousarray(np.asarray(x, dtype=np.float32))
    # sign(w), transposed to [dy, dx, ci, co]; +-1/0 are exact in bf16
    w1t = np.ascontiguousarray(
        np.sign(np.asarray(w1, np.float32)).transpose(2, 3, 1, 0)
    ).astype(ml_dtypes.bfloat16)
    w2t = np.ascontiguousarray(
        np.sign(np.asarray(w2, np.float32)).transpose(2, 3, 1, 0)
    ).astype(ml_dtypes.bfloat16)
    gb = np.stack(
        [np.asarray(v, np.float32)[c * 128:(c + 1) * 128]
         for v in (g1, b1, g2, b2) for c in range(CB)],
        axis=1,
    )
    # column order: g1_0 g1_1 b1_0 b1_1 g2_0 g2_1 b2_0 b2_1
    gb = np.ascontiguousarray(gb)
    in_maps = []
    for c in range(N_CORES):
        in_maps.append({
            "x": x[c * NL:(c + 1) * NL],
            "w1t": w1t,
            "w2t": w2t,
            "gb": gb,
        })
    return in_maps


def run(inputs, trace=False):
    """Run the kernel on 8 cores; returns (full_output, BassKernelResults)."""
    if "nc" not in _CACHE:
        _CACHE["nc"] = build()
    nc = _CACHE["nc"]
    in_maps = _prep_inputs(**inputs)
    res = bass_utils.run_bass_kernel_spmd(
        nc, in_maps, core_ids=list(range(N_CORES)), trace=trace)
    out = np.concatenate([res.results[c]["out"] for c in range(N_CORES)], axis=0)
    return out, res


def kernel(**inputs):
    out, _ = run(inputs, trace=False)
    return out
